# revision 13
# baseline (speedup 1.0000x reference)
"""MoE (top-2 of 5 experts, 3-layer MLP) Trainium2 Bass kernel.

Strategy: data-parallel over the 32768-token batch across 8 NeuronCores
(4096 tokens/core), expert weights replicated. All matmuls run in fp16
(PE full rate; fp16 products are exact in fp32 PSUM). The gating matmul
uses a 3-term fp16 split (x_hi@Wg_hi + x_hi@Wg_lo + x_lo@Wg_hi) whose
logit error (~4e-6) is far below the minimum top-2/3 score gap, so the
expert selection matches the fp32 reference exactly.

Per-core dataflow (groups of 512 tokens):
  - x arrives as host-prepared fp16 hi/lo planes, DMA-transposed on load
    into feature-major [128, 12, 512] tiles.
  - gating: feature-major logits [5, 512] in PSUM -> PE-transpose to
    token-major [128, 5] -> softmax (ACT Exp with accum denominator) ->
    masked top-2 -> renormalized gates -> per-token expert weights w.
  - experts: L1/L2 feature-major (weights stationary, ACT relu+bias
    eviction), L3 token-major (activations stationary) so the gate
    weight is a per-partition scalar: the PSUM eviction applies the
    gate and accumulates over experts (ACT copy for e=0, DVE
    scalar_tensor_tensor for e>0). Output is written token-major.
  - load-balance stats (sum of scores, top-2 counts) accumulate in a
    persistent PSUM bank via ones-vector matmuls; host combines them
    into lb_loss.
"""

import sys

if "/opt/trn_rl_repo" not in sys.path:
    sys.path.insert(0, "/opt/trn_rl_repo")

import numpy as np

B, DIN, E, H, O = 32768, 1536, 5, 512, 512
TEMPERATURE = 1.0
LB_WEIGHT = 0.01
NCORES = 8
BL = B // NCORES            # tokens per core
GT = 512                    # tokens per group
G = BL // GT                # groups per core
KO = DIN // 128             # k-chunks for DIN contraction
HC = H // 128               # feature chunks for H
TT = GT // 128              # 128-token tiles per group
NTILES = G * TT             # 128-token tiles per core


def build_kernel(use_b3: bool, reps: int = 1):
    import concourse.bass as bass
    import concourse.mybir as mybir
    import concourse.tile as tile
    from concourse import bacc
    from concourse.masks import make_identity

    dt = mybir.dt
    AF = mybir.ActivationFunctionType
    ALU = mybir.AluOpType

    nc = bacc.Bacc("TRN2", target_bir_lowering=False, debug=False)

    xh_d = nc.dram_tensor("xh", [BL, DIN], dt.float16, kind="ExternalInput").ap()
    xl_d = nc.dram_tensor("xl", [BL, DIN], dt.float16, kind="ExternalInput").ap()
    w1_d = nc.dram_tensor("w1", [E, DIN, H], dt.float16, kind="ExternalInput").ap()
    w2_d = nc.dram_tensor("w2", [E, H, H], dt.float16, kind="ExternalInput").ap()
    w3_d = nc.dram_tensor("w3", [E, H, O], dt.float16, kind="ExternalInput").ap()
    wgh_d = nc.dram_tensor("wgh", [DIN, E], dt.float16, kind="ExternalInput").ap()
    wgl_d = nc.dram_tensor("wgl", [DIN, E], dt.float16, kind="ExternalInput").ap()
    b1_d = nc.dram_tensor("b1r", [128, E, HC], dt.float32, kind="ExternalInput").ap()
    b2_d = nc.dram_tensor("b2r", [128, E, HC], dt.float32, kind="ExternalInput").ap()
    b3_d = nc.dram_tensor("b3", [E, O], dt.float16, kind="ExternalInput").ap()
    out_d = nc.dram_tensor("out", [BL, O], dt.float32, kind="ExternalOutput").ap()
    sc_d = nc.dram_tensor("scores", [BL, E], dt.float32, kind="ExternalOutput").ap()
    st_d = nc.dram_tensor("stats", [16, 1], dt.float32, kind="ExternalOutput").ap()

    with tile.TileContext(nc) as tc:
        with (
            tc.tile_pool(name="const", bufs=1) as cpool,
            tc.tile_pool(name="xt2", bufs=2) as xtpool,
            tc.tile_pool(name="xt1", bufs=1) as xtpool1,
            tc.tile_pool(name="act", bufs=2) as apool,
            tc.tile_pool(name="sm", bufs=4) as smpool,
            tc.tile_pool(name="wvp", bufs=2 * TT) as wvpool,
            tc.tile_pool(name="pmm", bufs=2, space="PSUM") as pmm,
            tc.tile_pool(name="pmisc", bufs=1, space="PSUM") as pmisc,
        ):
            # ---- constants / weights (resident) ----
            ident = cpool.tile([128, 128], dt.float32)
            make_identity(nc, ident[:])

            w1s, w2s, w3s = [], [], []
            for e in range(E):
                t1 = cpool.tile([128, KO, H], dt.float16, tag=f"w1_{e}")
                nc.sync.dma_start(t1[:], w1_d[e].rearrange("(ko p) h -> p ko h", p=128))
                w1s.append(t1)
                t2 = cpool.tile([128, HC, H], dt.float16, tag=f"w2_{e}")
                nc.sync.dma_start(t2[:], w2_d[e].rearrange("(k p) h -> p k h", p=128))
                w2s.append(t2)
                t3 = cpool.tile([128, HC, O], dt.float16, tag=f"w3_{e}")
                nc.sync.dma_start(t3[:], w3_d[e].rearrange("(k p) o -> p k o", p=128))
                w3s.append(t3)
            wgh = cpool.tile([128, KO, E], dt.float16)
            nc.sync.dma_start(wgh[:], wgh_d.rearrange("(ko p) e -> p ko e", p=128))
            wgl = cpool.tile([128, KO, E], dt.float16)
            nc.sync.dma_start(wgl[:], wgl_d.rearrange("(ko p) e -> p ko e", p=128))
            b1s = cpool.tile([128, E, HC], dt.float32)
            nc.sync.dma_start(b1s[:], b1_d)
            b2s = cpool.tile([128, E, HC], dt.float32)
            nc.sync.dma_start(b2s[:], b2_d)
            if use_b3:
                b3s = cpool.tile([E, O], dt.float16)
                nc.sync.dma_start(b3s[:], b3_d)
                onesrow = cpool.tile([1, 128], dt.float16)
                nc.vector.memset(onesrow[:], 1.0)

            scstage = cpool.tile([128, NTILES, E], dt.float32)
            sacc = cpool.tile([128, 2 * E], dt.float32)

            for rep in range(reps):
                nc.vector.memset(sacc[:], 0.0)
                for g in range(G):
                    rows = slice(g * GT, (g + 1) * GT)
                    xthi = xtpool.tile([128, KO, GT], dt.float16, tag="xthi")
                    nc.sync.dma_start_transpose(xthi[:], xh_d[rows, :])
                    xtlo = xtpool1.tile([128, KO, GT], dt.float16, tag="xtlo")
                    nc.sync.dma_start_transpose(xtlo[:], xl_d[rows, :])

                    # ---- gating: logits_T [5, GT] = Wg.T @ x.T (3-term fp16 split)
                    pl = pmisc.tile([E, GT], dt.float32, tag="pl")
                    terms = [(wgh, xthi), (wgh, xtlo), (wgl, xthi)]
                    n_mm = len(terms) * KO
                    i = 0
                    for wt, xt in terms:
                        for ko in range(KO):
                            nc.tensor.matmul(
                                pl[:], wt[:, ko, :], xt[:, ko, :],
                                start=(i == 0), stop=(i == n_mm - 1),
                            )
                            i += 1
                    lsb = smpool.tile([E, GT], dt.float32, tag="lsb")
                    nc.scalar.mul(lsb[:], pl[:], 1.0 / TEMPERATURE)

                    wvs = []
                    for t in range(TT):
                        tile_idx = g * TT + t
                        tsl = slice(t * 128, (t + 1) * 128)
                        ptt = pmisc.tile([128, E], dt.float32, tag="ptt")
                        nc.tensor.transpose(ptt[:], lsb[:, tsl], ident[:E, :E])
                        nmx = smpool.tile([128, 1], dt.float32, tag="nmx")
                        nc.vector.tensor_reduce(
                            nmx[:], ptt[:], axis=mybir.AxisListType.X,
                            op=ALU.max, negate=True,
                        )
                        es = smpool.tile([128, E], dt.float32, tag="es")
                        se = smpool.tile([128, 1], dt.float32, tag="se")
                        nc.scalar.activation(
                            es[:], ptt[:], AF.Exp, bias=nmx[:], scale=1.0,
                            accum_out=se[:],
                        )
                        rse = smpool.tile([128, 1], dt.float32, tag="rse")
                        nc.vector.reciprocal(rse[:], se[:])
                        sc = scstage[:, tile_idx, :]
                        nc.vector.tensor_scalar_mul(sc, es[:], rse[:])
                        # top-2 by masked max
                        s1 = smpool.tile([128, 1], dt.float32, tag="s1")
                        nc.vector.tensor_reduce(
                            s1[:], sc, axis=mybir.AxisListType.X, op=ALU.max)
                        m1 = smpool.tile([128, E], dt.float32, tag="m1")
                        nc.vector.tensor_scalar(m1[:], sc, s1[:], None, ALU.is_equal)
                        s2m = smpool.tile([128, E], dt.float32, tag="s2m")
                        nc.vector.scalar_tensor_tensor(
                            s2m[:], m1[:], -1e30, sc, op0=ALU.mult, op1=ALU.add)
                        s2 = smpool.tile([128, 1], dt.float32, tag="s2")
                        nc.vector.tensor_reduce(
                            s2[:], s2m[:], axis=mybir.AxisListType.X, op=ALU.max)
                        m2 = smpool.tile([128, E], dt.float32, tag="m2")
                        nc.vector.tensor_scalar(m2[:], s2m[:], s2[:], None, ALU.is_equal)
                        den = smpool.tile([128, 1], dt.float32, tag="den")
                        nc.vector.tensor_add(den[:], s1[:], s2[:])
                        rden = smpool.tile([128, 1], dt.float32, tag="rden")
                        nc.vector.reciprocal(rden[:], den[:])
                        g1 = smpool.tile([128, 1], dt.float32, tag="g1")
                        nc.vector.tensor_mul(g1[:], s1[:], rden[:])
                        g2 = smpool.tile([128, 1], dt.float32, tag="g2")
                        nc.vector.tensor_mul(g2[:], s2[:], rden[:])
                        wv = wvpool.tile([128, E], dt.float32, tag="wv")
                        nc.vector.tensor_scalar_mul(wv[:], m2[:], g2[:])
                        nc.vector.scalar_tensor_tensor(
                            wv[:], m1[:], g1[:], wv[:], op0=ALU.mult, op1=ALU.add)
                        wvs.append(wv)
                        m12 = smpool.tile([128, E], dt.float32, tag="m12")
                        nc.vector.tensor_add(m12[:], m1[:], m2[:])
                        # per-partition stats accumulation (RAW-chained on DVE;
                        # cross-tile PSUM-matmul accumulation is unsafe under
                        # scheduler reordering)
                        nc.vector.tensor_add(sacc[:, 0:E], sacc[:, 0:E], sc)
                        nc.vector.tensor_add(sacc[:, E:2 * E], sacc[:, E:2 * E], m12[:])

                    # ---- experts ----
                    acc = apool.tile([128, TT, O], dt.float32, tag="acc")
                    for e in range(E):
                        h1 = apool.tile([128, HC, GT], dt.float16, tag="h1")
                        for hc in range(HC):
                            ph = pmm.tile([128, GT], dt.float32, tag="ph")
                            for ko in range(KO):
                                nc.tensor.matmul(
                                    ph[:], w1s[e][:, ko, hc * 128:(hc + 1) * 128],
                                    xthi[:, ko, :],
                                    start=(ko == 0), stop=(ko == KO - 1))
                            nc.scalar.activation(
                                h1[:, hc, :], ph[:], AF.Relu,
                                bias=b1s[:, e, hc:hc + 1], scale=1.0)
                        h2 = apool.tile([128, HC, GT], dt.float16, tag="h2")
                        for oc in range(HC):
                            ph = pmm.tile([128, GT], dt.float32, tag="ph")
                            for k in range(HC):
                                nc.tensor.matmul(
                                    ph[:], w2s[e][:, k, oc * 128:(oc + 1) * 128],
                                    h1[:, k, :],
                                    start=(k == 0), stop=(k == HC - 1))
                            nc.scalar.activation(
                                h2[:, oc, :], ph[:], AF.Relu,
                                bias=b2s[:, e, oc:oc + 1], scale=1.0)
                        for t in range(TT):
                            tsl = slice(t * 128, (t + 1) * 128)
                            po = pmm.tile([128, O], dt.float32, tag="po")
                            for k in range(HC):
                                nc.tensor.matmul(
                                    po[:], h2[:, k, tsl], w3s[e][:, k, :],
                                    start=(k == 0),
                                    stop=(k == HC - 1 and not use_b3))
                            if use_b3:
                                nc.tensor.matmul(
                                    po[:], onesrow[:], b3s[e:e + 1, :],
                                    start=False, stop=True)
                            wsl = wvs[t][:, e:e + 1]
                            if e == 0:
                                nc.scalar.activation(
                                    acc[:, t, :], po[:], AF.Copy,
                                    bias=0.0, scale=wsl)
                            else:
                                nc.vector.scalar_tensor_tensor(
                                    acc[:, t, :], po[:], wsl, acc[:, t, :],
                                    op0=ALU.mult, op1=ALU.add)

                    nc.sync.dma_start(
                        out_d.rearrange("(g t p) o -> g p t o", t=TT, p=128)[g],
                        acc[:])

                # ---- epilogue (per rep; cheap): partition-reduce the stats
                pst = pmisc.tile([2 * E, 128], dt.float32, tag="pst")
                nc.tensor.transpose(pst[:], sacc[:], ident[:])
                stsb = smpool.tile([2 * E, 128], dt.float32, tag="stsb")
                nc.scalar.copy(stsb[:], pst[:])
                stq = smpool.tile([2 * E, 1], dt.float32, tag="stq")
                nc.vector.tensor_reduce(
                    stq[:], stsb[:], axis=mybir.AxisListType.X, op=ALU.add)
                nc.sync.dma_start(st_d[:2 * E], stq[:])
                nc.sync.dma_start(
                    sc_d.rearrange("(n p) e -> p n e", p=128), scstage[:])

    nc.compile()
    return nc


class _SpmdRunner:
    """Compile once; run the SPMD kernel on 8 cores with sharded inputs."""

    def __init__(self, nc, n_cores=NCORES):
        import jax
        import concourse.mybir as mybir
        from jax.sharding import Mesh, PartitionSpec, NamedSharding
        from jax.experimental.shard_map import shard_map
        from concourse import bass2jax
        from concourse.bass2jax import _bass_exec_p, install_neuronx_cc_hook

        install_neuronx_cc_hook()
        self.jax = jax
        self.nc = nc
        self.n_cores = n_cores
        in_names, out_names, out_avals = [], [], []
        partition_name = (
            nc.partition_id_tensor.name if nc.partition_id_tensor else None)
        for alloc in nc.m.functions[0].allocations:
            if not isinstance(alloc, mybir.MemoryLocationSet):
                continue
            name = alloc.memorylocations[0].name
            if alloc.kind == "ExternalInput":
                if name != partition_name:
                    in_names.append(name)
            elif alloc.kind == "ExternalOutput":
                out_names.append(name)
                out_avals.append(jax.core.ShapedArray(
                    tuple(alloc.tensor_shape), mybir.dt.np(alloc.dtype)))
        self.in_names, self.out_names, self.out_avals = (
            in_names, out_names, out_avals)

        n_params = len(in_names)
        all_in_names = list(in_names) + list(out_names)
        if partition_name is not None:
            all_in_names.append(partition_name)

        def _body(*args):
            operands = list(args)
            if partition_name is not None:
                operands.append(bass2jax.partition_id_tensor())
            outs = _bass_exec_p.bind(
                *operands,
                out_avals=tuple(out_avals),
                in_names=tuple(all_in_names),
                out_names=tuple(out_names),
                lowering_input_output_aliases=(),
                sim_require_finite=True,
                sim_require_nnan=True,
                nc=nc,
            )
            return tuple(outs)

        devices = jax.devices()[:n_cores]
        mesh = Mesh(np.asarray(devices), ("core",))
        spec = PartitionSpec("core")
        self.sharding = NamedSharding(mesh, spec)
        self.fn = jax.jit(
            shard_map(_body, mesh=mesh, in_specs=(spec,) * (n_params + len(out_names)),
                      out_specs=(spec,) * len(out_names), check_rep=False),
            keep_unused=True,
        )
        import jax.numpy as jnp
        zero_shapes = [
            ((n_cores * av.shape[0],) + tuple(av.shape[1:]), av.dtype)
            for av in out_avals]

        def _zeros():
            return tuple(jnp.zeros(s, d) for s, d in zero_shapes)

        self.zfn = jax.jit(_zeros, out_shardings=(spec and self.sharding,) * len(zero_shapes))

    def put_inputs(self, in_maps):
        arrs = []
        for name in self.in_names:
            cat = np.concatenate([m[name] for m in in_maps], axis=0)
            arrs.append(self.jax.device_put(cat, self.sharding))
        return arrs

    def run(self, dev_inputs):
        return self.fn(*dev_inputs, *self.zfn())

    def results(self, outs):
        res = [dict() for _ in range(self.n_cores)]
        for i, name in enumerate(self.out_names):
            g = np.asarray(outs[i]).reshape(
                self.n_cores, *self.out_avals[i].shape)
            for c in range(self.n_cores):
                res[c][name] = g[c]
        return res


_RUNNERS = {}


def _get_runner(use_b3: bool, reps: int = 1):
    key = (use_b3, reps)
    if key not in _RUNNERS:
        nc = build_kernel(use_b3, reps)
        _RUNNERS[key] = _SpmdRunner(nc)
    return _RUNNERS[key]


def _prep_inputs(x, Wg, W1, b1, W2, b2, W3, b3):
    f16, f32 = np.float16, np.float32
    xh = x.astype(f16)
    xl = (x - xh.astype(f32)).astype(f16)
    wgh = Wg.astype(f16)
    wgl = (Wg - wgh.astype(f32)).astype(f16)
    w1h, w2h, w3h = W1.astype(f16), W2.astype(f16), W3.astype(f16)
    b1r = np.ascontiguousarray(
        b1.reshape(E, HC, 128).transpose(2, 0, 1).astype(f32))
    b2r = np.ascontiguousarray(
        b2.reshape(E, HC, 128).transpose(2, 0, 1).astype(f32))
    b3h = b3.astype(f16)
    in_maps = []
    for c in range(NCORES):
        rows = slice(c * BL, (c + 1) * BL)
        in_maps.append({
            "xh": np.ascontiguousarray(xh[rows]),
            "xl": np.ascontiguousarray(xl[rows]),
            "w1": w1h, "w2": w2h, "w3": w3h,
            "wgh": wgh, "wgl": wgl,
            "b1r": b1r, "b2r": b2r, "b3": b3h,
        })
    return in_maps


def kernel(x, Wg, W1, b1, W2, b2, W3, b3, top_k):
    x = np.asarray(x, np.float32)
    Wg = np.asarray(Wg, np.float32)
    W1 = np.asarray(W1, np.float32)
    b1 = np.asarray(b1, np.float32)
    W2 = np.asarray(W2, np.float32)
    b2 = np.asarray(b2, np.float32)
    W3 = np.asarray(W3, np.float32)
    b3 = np.asarray(b3, np.float32)
    assert int(top_k) == 2, "kernel hardcodes top_k=2"

    use_b3 = bool(np.any(b3))
    runner = _get_runner(use_b3)
    in_maps = _prep_inputs(x, Wg, W1, b1, W2, b2, W3, b3)
    dev_in = runner.put_inputs(in_maps)
    outs = runner.run(dev_in)
    res = runner.results(outs)

    out = np.concatenate([res[c]["out"] for c in range(NCORES)], axis=0)
    scores = np.concatenate([res[c]["scores"] for c in range(NCORES)], axis=0)
    stats = np.sum([res[c]["stats"][:, 0] for c in range(NCORES)], axis=0)
    mean_prob = stats[0:E] / B
    frac = stats[E:2 * E] / B
    lb_loss = np.float32(LB_WEIGHT * E * np.dot(mean_prob, frac))
    return out, lb_loss, scores


# revision 15
# speedup vs baseline: 13.5869x; 13.5869x over previous
"""MoE (top-2 of 5 experts, 3-layer MLP) Trainium2 Bass kernel.

Strategy: data-parallel over the 32768-token batch across 8 NeuronCores
(4096 tokens/core), expert weights replicated. All matmuls run in fp16
(PE full rate; fp16 products are exact in fp32 PSUM). The gating matmul
uses a 3-term fp16 split (x_hi@Wg_hi + x_hi@Wg_lo + x_lo@Wg_hi) whose
logit error (~4e-6) is far below the minimum top-2/3 score gap, so the
expert selection matches the fp32 reference exactly.

Per-core dataflow (groups of 512 tokens):
  - x arrives as host-prepared fp16 hi/lo planes, DMA-transposed on load
    into feature-major [128, 12, 512] tiles.
  - gating: feature-major logits [5, 512] in PSUM -> PE-transpose to
    token-major [128, 5] -> softmax (ACT Exp with accum denominator) ->
    masked top-2 -> renormalized gates -> per-token expert weights w.
  - experts: L1/L2 feature-major (weights stationary, ACT relu+bias
    eviction), L3 token-major (activations stationary) so the gate
    weight is a per-partition scalar: the PSUM eviction applies the
    gate and accumulates over experts (ACT copy for e=0, DVE
    scalar_tensor_tensor for e>0). Output is written token-major.
  - load-balance stats (sum of scores, top-2 counts) accumulate in a
    persistent PSUM bank via ones-vector matmuls; host combines them
    into lb_loss.
"""

import sys

if "/opt/trn_rl_repo" not in sys.path:
    sys.path.insert(0, "/opt/trn_rl_repo")

import numpy as np

B, DIN, E, H, O = 32768, 1536, 5, 512, 512
TEMPERATURE = 1.0
LB_WEIGHT = 0.01
NCORES = 8
BL = B // NCORES            # tokens per core
GT = 512                    # tokens per group
G = BL // GT                # groups per core
KO = DIN // 128             # k-chunks for DIN contraction
HC = H // 128               # feature chunks for H
TT = GT // 128              # 128-token tiles per group
NTILES = G * TT             # 128-token tiles per core


def build_kernel(use_b3: bool, reps: int = 1):
    import concourse.bass as bass
    import concourse.mybir as mybir
    import concourse.tile as tile
    from concourse import bacc
    from concourse.masks import make_identity

    dt = mybir.dt
    AF = mybir.ActivationFunctionType
    ALU = mybir.AluOpType

    nc = bacc.Bacc("TRN2", target_bir_lowering=False, debug=False)

    xh_d = nc.dram_tensor("xh", [BL, DIN], dt.float16, kind="ExternalInput").ap()
    xl_d = nc.dram_tensor("xl", [BL, DIN], dt.float16, kind="ExternalInput").ap()
    w1_d = nc.dram_tensor("w1", [E, DIN, H], dt.float16, kind="ExternalInput").ap()
    w2_d = nc.dram_tensor("w2", [E, H, H], dt.float16, kind="ExternalInput").ap()
    w3_d = nc.dram_tensor("w3", [E, H, O], dt.float16, kind="ExternalInput").ap()
    wgh_d = nc.dram_tensor("wgh", [DIN, E], dt.float16, kind="ExternalInput").ap()
    wgl_d = nc.dram_tensor("wgl", [DIN, E], dt.float16, kind="ExternalInput").ap()
    b1_d = nc.dram_tensor("b1r", [128, E, HC], dt.float32, kind="ExternalInput").ap()
    b2_d = nc.dram_tensor("b2r", [128, E, HC], dt.float32, kind="ExternalInput").ap()
    b3_d = nc.dram_tensor("b3", [E, O], dt.float16, kind="ExternalInput").ap()
    out_d = nc.dram_tensor("out", [BL, O], dt.float32, kind="ExternalOutput").ap()
    sc_d = nc.dram_tensor("scores", [BL, E], dt.float32, kind="ExternalOutput").ap()
    st_d = nc.dram_tensor("stats", [16, 1], dt.float32, kind="ExternalOutput").ap()

    with tile.TileContext(nc) as tc:
        with (
            tc.tile_pool(name="const", bufs=1) as cpool,
            tc.tile_pool(name="xt2", bufs=2) as xtpool,
            tc.tile_pool(name="xt1", bufs=1) as xtpool1,
            tc.tile_pool(name="act", bufs=2) as apool,
            tc.tile_pool(name="sm", bufs=4) as smpool,
            tc.tile_pool(name="wvp", bufs=2 * TT) as wvpool,
            tc.tile_pool(name="pmm", bufs=2, space="PSUM") as pmm,
            tc.tile_pool(name="pmisc", bufs=1, space="PSUM") as pmisc,
        ):
            # ---- constants / weights (resident) ----
            ident = cpool.tile([128, 128], dt.float32)
            make_identity(nc, ident[:])

            w1s, w2s, w3s = [], [], []
            for e in range(E):
                t1 = cpool.tile([128, KO, H], dt.float16, tag=f"w1_{e}")
                nc.sync.dma_start(t1[:], w1_d[e].rearrange("(ko p) h -> p ko h", p=128))
                w1s.append(t1)
                t2 = cpool.tile([128, HC, H], dt.float16, tag=f"w2_{e}")
                nc.sync.dma_start(t2[:], w2_d[e].rearrange("(k p) h -> p k h", p=128))
                w2s.append(t2)
                t3 = cpool.tile([128, HC, O], dt.float16, tag=f"w3_{e}")
                nc.sync.dma_start(t3[:], w3_d[e].rearrange("(k p) o -> p k o", p=128))
                w3s.append(t3)
            wgh = cpool.tile([128, KO, E], dt.float16)
            nc.sync.dma_start(wgh[:], wgh_d.rearrange("(ko p) e -> p ko e", p=128))
            wgl = cpool.tile([128, KO, E], dt.float16)
            nc.sync.dma_start(wgl[:], wgl_d.rearrange("(ko p) e -> p ko e", p=128))
            b1s = cpool.tile([128, E, HC], dt.float32)
            nc.sync.dma_start(b1s[:], b1_d)
            b2s = cpool.tile([128, E, HC], dt.float32)
            nc.sync.dma_start(b2s[:], b2_d)
            if use_b3:
                b3s = cpool.tile([E, O], dt.float16)
                nc.sync.dma_start(b3s[:], b3_d)
                onesrow = cpool.tile([1, 128], dt.float16)
                nc.vector.memset(onesrow[:], 1.0)

            scstage = cpool.tile([128, NTILES, E], dt.float32)
            sacc = cpool.tile([128, 2 * E], dt.float32)

            def emit_rep():
                nc.vector.memset(sacc[:], 0.0)
                for g in range(G):
                    rows = slice(g * GT, (g + 1) * GT)
                    xthi = xtpool.tile([128, KO, GT], dt.float16, tag="xthi")
                    nc.sync.dma_start_transpose(xthi[:], xh_d[rows, :])
                    xtlo = xtpool1.tile([128, KO, GT], dt.float16, tag="xtlo")
                    nc.sync.dma_start_transpose(xtlo[:], xl_d[rows, :])

                    # ---- gating: logits_T [5, GT] = Wg.T @ x.T (3-term fp16 split)
                    pl = pmisc.tile([E, GT], dt.float32, tag="pl")
                    terms = [(wgh, xthi), (wgh, xtlo), (wgl, xthi)]
                    n_mm = len(terms) * KO
                    i = 0
                    for wt, xt in terms:
                        for ko in range(KO):
                            nc.tensor.matmul(
                                pl[:], wt[:, ko, :], xt[:, ko, :],
                                start=(i == 0), stop=(i == n_mm - 1),
                            )
                            i += 1
                    lsb = smpool.tile([E, GT], dt.float32, tag="lsb")
                    nc.scalar.mul(lsb[:], pl[:], 1.0 / TEMPERATURE)

                    wvs = []
                    for t in range(TT):
                        tile_idx = g * TT + t
                        tsl = slice(t * 128, (t + 1) * 128)
                        ptt = pmisc.tile([128, E], dt.float32, tag="ptt")
                        nc.tensor.transpose(ptt[:], lsb[:, tsl], ident[:E, :E])
                        nmx = smpool.tile([128, 1], dt.float32, tag="nmx")
                        nc.vector.tensor_reduce(
                            nmx[:], ptt[:], axis=mybir.AxisListType.X,
                            op=ALU.max, negate=True,
                        )
                        es = smpool.tile([128, E], dt.float32, tag="es")
                        se = smpool.tile([128, 1], dt.float32, tag="se")
                        nc.scalar.activation(
                            es[:], ptt[:], AF.Exp, bias=nmx[:], scale=1.0,
                            accum_out=se[:],
                        )
                        rse = smpool.tile([128, 1], dt.float32, tag="rse")
                        nc.vector.reciprocal(rse[:], se[:])
                        sc = scstage[:, tile_idx, :]
                        nc.vector.tensor_scalar_mul(sc, es[:], rse[:])
                        # top-2 by masked max
                        s1 = smpool.tile([128, 1], dt.float32, tag="s1")
                        nc.vector.tensor_reduce(
                            s1[:], sc, axis=mybir.AxisListType.X, op=ALU.max)
                        m1 = smpool.tile([128, E], dt.float32, tag="m1")
                        nc.vector.tensor_scalar(m1[:], sc, s1[:], None, ALU.is_equal)
                        s2m = smpool.tile([128, E], dt.float32, tag="s2m")
                        nc.vector.scalar_tensor_tensor(
                            s2m[:], m1[:], -1e30, sc, op0=ALU.mult, op1=ALU.add)
                        s2 = smpool.tile([128, 1], dt.float32, tag="s2")
                        nc.vector.tensor_reduce(
                            s2[:], s2m[:], axis=mybir.AxisListType.X, op=ALU.max)
                        m2 = smpool.tile([128, E], dt.float32, tag="m2")
                        nc.vector.tensor_scalar(m2[:], s2m[:], s2[:], None, ALU.is_equal)
                        den = smpool.tile([128, 1], dt.float32, tag="den")
                        nc.vector.tensor_add(den[:], s1[:], s2[:])
                        rden = smpool.tile([128, 1], dt.float32, tag="rden")
                        nc.vector.reciprocal(rden[:], den[:])
                        g1 = smpool.tile([128, 1], dt.float32, tag="g1")
                        nc.vector.tensor_mul(g1[:], s1[:], rden[:])
                        g2 = smpool.tile([128, 1], dt.float32, tag="g2")
                        nc.vector.tensor_mul(g2[:], s2[:], rden[:])
                        wv = wvpool.tile([128, E], dt.float32, tag="wv")
                        nc.vector.tensor_scalar_mul(wv[:], m2[:], g2[:])
                        nc.vector.scalar_tensor_tensor(
                            wv[:], m1[:], g1[:], wv[:], op0=ALU.mult, op1=ALU.add)
                        wvs.append(wv)
                        m12 = smpool.tile([128, E], dt.float32, tag="m12")
                        nc.vector.tensor_add(m12[:], m1[:], m2[:])
                        # per-partition stats accumulation (RAW-chained on DVE;
                        # cross-tile PSUM-matmul accumulation is unsafe under
                        # scheduler reordering)
                        nc.vector.tensor_add(sacc[:, 0:E], sacc[:, 0:E], sc)
                        nc.vector.tensor_add(sacc[:, E:2 * E], sacc[:, E:2 * E], m12[:])

                    # ---- experts ----
                    acc = apool.tile([128, TT, O], dt.float32, tag="acc")
                    for e in range(E):
                        h1 = apool.tile([128, HC, GT], dt.float16, tag="h1")
                        for hc in range(HC):
                            ph = pmm.tile([128, GT], dt.float32, tag="ph")
                            for ko in range(KO):
                                nc.tensor.matmul(
                                    ph[:], w1s[e][:, ko, hc * 128:(hc + 1) * 128],
                                    xthi[:, ko, :],
                                    start=(ko == 0), stop=(ko == KO - 1))
                            nc.scalar.activation(
                                h1[:, hc, :], ph[:], AF.Relu,
                                bias=b1s[:, e, hc:hc + 1], scale=1.0)
                        h2 = apool.tile([128, HC, GT], dt.float16, tag="h2")
                        for oc in range(HC):
                            ph = pmm.tile([128, GT], dt.float32, tag="ph")
                            for k in range(HC):
                                nc.tensor.matmul(
                                    ph[:], w2s[e][:, k, oc * 128:(oc + 1) * 128],
                                    h1[:, k, :],
                                    start=(k == 0), stop=(k == HC - 1))
                            nc.scalar.activation(
                                h2[:, oc, :], ph[:], AF.Relu,
                                bias=b2s[:, e, oc:oc + 1], scale=1.0)
                        for t in range(TT):
                            tsl = slice(t * 128, (t + 1) * 128)
                            po = pmm.tile([128, O], dt.float32, tag="po")
                            for k in range(HC):
                                nc.tensor.matmul(
                                    po[:], h2[:, k, tsl], w3s[e][:, k, :],
                                    start=(k == 0),
                                    stop=(k == HC - 1 and not use_b3))
                            if use_b3:
                                nc.tensor.matmul(
                                    po[:], onesrow[:], b3s[e:e + 1, :],
                                    start=False, stop=True)
                            wsl = wvs[t][:, e:e + 1]
                            if e == 0:
                                nc.scalar.activation(
                                    acc[:, t, :], po[:], AF.Copy,
                                    bias=0.0, scale=wsl)
                            else:
                                nc.vector.scalar_tensor_tensor(
                                    acc[:, t, :], po[:], wsl, acc[:, t, :],
                                    op0=ALU.mult, op1=ALU.add)

                    nc.sync.dma_start(
                        out_d.rearrange("(g t p) o -> g p t o", t=TT, p=128)[g],
                        acc[:])

                # ---- epilogue (per rep; cheap): partition-reduce the stats
                pst = pmisc.tile([2 * E, 128], dt.float32, tag="pst")
                nc.tensor.transpose(pst[:], sacc[:], ident[:])
                stsb = smpool.tile([2 * E, 128], dt.float32, tag="stsb")
                nc.scalar.copy(stsb[:], pst[:])
                stq = smpool.tile([2 * E, 1], dt.float32, tag="stq")
                nc.vector.tensor_reduce(
                    stq[:], stsb[:], axis=mybir.AxisListType.X, op=ALU.add)
                nc.sync.dma_start(st_d[:2 * E], stq[:])
                nc.sync.dma_start(
                    sc_d.rearrange("(n p) e -> p n e", p=128), scstage[:])

            if reps == 1:
                emit_rep()
            else:
                with tc.For_i(0, reps, 1):
                    emit_rep()

    nc.compile()
    return nc


class _SpmdRunner:
    """Compile once; run the SPMD kernel on 8 cores with sharded inputs."""

    def __init__(self, nc, n_cores=NCORES):
        import jax
        import concourse.mybir as mybir
        from jax.sharding import Mesh, PartitionSpec, NamedSharding
        from jax.experimental.shard_map import shard_map
        from concourse import bass2jax
        from concourse.bass2jax import _bass_exec_p, install_neuronx_cc_hook

        install_neuronx_cc_hook()
        self.jax = jax
        self.nc = nc
        self.n_cores = n_cores
        in_names, out_names, out_avals = [], [], []
        partition_name = (
            nc.partition_id_tensor.name if nc.partition_id_tensor else None)
        for alloc in nc.m.functions[0].allocations:
            if not isinstance(alloc, mybir.MemoryLocationSet):
                continue
            name = alloc.memorylocations[0].name
            if alloc.kind == "ExternalInput":
                if name != partition_name:
                    in_names.append(name)
            elif alloc.kind == "ExternalOutput":
                out_names.append(name)
                out_avals.append(jax.core.ShapedArray(
                    tuple(alloc.tensor_shape), mybir.dt.np(alloc.dtype)))
        self.in_names, self.out_names, self.out_avals = (
            in_names, out_names, out_avals)

        n_params = len(in_names)
        all_in_names = list(in_names) + list(out_names)
        if partition_name is not None:
            all_in_names.append(partition_name)

        def _body(*args):
            operands = list(args)
            if partition_name is not None:
                operands.append(bass2jax.partition_id_tensor())
            outs = _bass_exec_p.bind(
                *operands,
                out_avals=tuple(out_avals),
                in_names=tuple(all_in_names),
                out_names=tuple(out_names),
                lowering_input_output_aliases=(),
                sim_require_finite=True,
                sim_require_nnan=True,
                nc=nc,
            )
            return tuple(outs)

        devices = jax.devices()[:n_cores]
        mesh = Mesh(np.asarray(devices), ("core",))
        spec = PartitionSpec("core")
        self.sharding = NamedSharding(mesh, spec)
        self.fn = jax.jit(
            shard_map(_body, mesh=mesh, in_specs=(spec,) * (n_params + len(out_names)),
                      out_specs=(spec,) * len(out_names), check_rep=False),
            keep_unused=True,
        )
        import jax.numpy as jnp
        zero_shapes = [
            ((n_cores * av.shape[0],) + tuple(av.shape[1:]), av.dtype)
            for av in out_avals]

        def _zeros():
            return tuple(jnp.zeros(s, d) for s, d in zero_shapes)

        self.zfn = jax.jit(_zeros, out_shardings=(spec and self.sharding,) * len(zero_shapes))

    def put_inputs(self, in_maps):
        arrs = []
        for name in self.in_names:
            cat = np.concatenate([m[name] for m in in_maps], axis=0)
            arrs.append(self.jax.device_put(cat, self.sharding))
        return arrs

    def run(self, dev_inputs):
        return self.fn(*dev_inputs, *self.zfn())

    def results(self, outs):
        res = [dict() for _ in range(self.n_cores)]
        for i, name in enumerate(self.out_names):
            g = np.asarray(outs[i]).reshape(
                self.n_cores, *self.out_avals[i].shape)
            for c in range(self.n_cores):
                res[c][name] = g[c]
        return res


_RUNNERS = {}


def _get_runner(use_b3: bool, reps: int = 1):
    key = (use_b3, reps)
    if key not in _RUNNERS:
        nc = build_kernel(use_b3, reps)
        _RUNNERS[key] = _SpmdRunner(nc)
    return _RUNNERS[key]


def _prep_inputs(x, Wg, W1, b1, W2, b2, W3, b3):
    f16, f32 = np.float16, np.float32
    xh = x.astype(f16)
    xl = (x - xh.astype(f32)).astype(f16)
    wgh = Wg.astype(f16)
    wgl = (Wg - wgh.astype(f32)).astype(f16)
    w1h, w2h, w3h = W1.astype(f16), W2.astype(f16), W3.astype(f16)
    b1r = np.ascontiguousarray(
        b1.reshape(E, HC, 128).transpose(2, 0, 1).astype(f32))
    b2r = np.ascontiguousarray(
        b2.reshape(E, HC, 128).transpose(2, 0, 1).astype(f32))
    b3h = b3.astype(f16)
    in_maps = []
    for c in range(NCORES):
        rows = slice(c * BL, (c + 1) * BL)
        in_maps.append({
            "xh": np.ascontiguousarray(xh[rows]),
            "xl": np.ascontiguousarray(xl[rows]),
            "w1": w1h, "w2": w2h, "w3": w3h,
            "wgh": wgh, "wgl": wgl,
            "b1r": b1r, "b2r": b2r, "b3": b3h,
        })
    return in_maps


def kernel(x, Wg, W1, b1, W2, b2, W3, b3, top_k):
    x = np.asarray(x, np.float32)
    Wg = np.asarray(Wg, np.float32)
    W1 = np.asarray(W1, np.float32)
    b1 = np.asarray(b1, np.float32)
    W2 = np.asarray(W2, np.float32)
    b2 = np.asarray(b2, np.float32)
    W3 = np.asarray(W3, np.float32)
    b3 = np.asarray(b3, np.float32)
    assert int(top_k) == 2, "kernel hardcodes top_k=2"

    use_b3 = bool(np.any(b3))
    runner = _get_runner(use_b3)
    in_maps = _prep_inputs(x, Wg, W1, b1, W2, b2, W3, b3)
    dev_in = runner.put_inputs(in_maps)
    outs = runner.run(dev_in)
    res = runner.results(outs)

    out = np.concatenate([res[c]["out"] for c in range(NCORES)], axis=0)
    scores = np.concatenate([res[c]["scores"] for c in range(NCORES)], axis=0)
    stats = np.sum([res[c]["stats"][:, 0] for c in range(NCORES)], axis=0)
    mean_prob = stats[0:E] / B
    frac = stats[E:2 * E] / B
    lb_loss = np.float32(LB_WEIGHT * E * np.dot(mean_prob, frac))
    return out, lb_loss, scores


# revision 21
# speedup vs baseline: 13.8093x; 1.0164x over previous
"""MoE (top-2 of 5 experts, 3-layer MLP) Trainium2 Bass kernel.

Strategy: data-parallel over the 32768-token batch across 8 NeuronCores
(4096 tokens/core), expert weights replicated. All matmuls run in fp16
(PE full rate; fp16 products are exact in fp32 PSUM). The gating matmul
uses a 3-term fp16 split (x_hi@Wg_hi + x_hi@Wg_lo + x_lo@Wg_hi) whose
logit error (~4e-6) is far below the minimum top-2/3 score gap, so the
expert selection matches the fp32 reference exactly.

Per-core dataflow (groups of 512 tokens):
  - x arrives as host-prepared fp16 hi/lo planes, DMA-transposed on load
    into feature-major [128, 12, 512] tiles.
  - gating: feature-major logits [5, 512] in PSUM -> PE-transpose to
    token-major [128, 5] -> softmax (ACT Exp with accum denominator) ->
    masked top-2 -> renormalized gates -> per-token expert weights w.
  - experts: L1/L2 feature-major (weights stationary, ACT relu+bias
    eviction), L3 token-major (activations stationary) so the gate
    weight is a per-partition scalar: the PSUM eviction applies the
    gate and accumulates over experts (ACT copy for e=0, DVE
    scalar_tensor_tensor for e>0). Output is written token-major.
  - load-balance stats (sum of scores, top-2 counts) accumulate in a
    persistent PSUM bank via ones-vector matmuls; host combines them
    into lb_loss.
"""

import sys

if "/opt/trn_rl_repo" not in sys.path:
    sys.path.insert(0, "/opt/trn_rl_repo")

import numpy as np

B, DIN, E, H, O = 32768, 1536, 5, 512, 512
TEMPERATURE = 1.0
LB_WEIGHT = 0.01
NCORES = 8
BL = B // NCORES            # tokens per core
GT = 512                    # tokens per group
G = BL // GT                # groups per core
KO = DIN // 128             # k-chunks for DIN contraction
HC = H // 128               # feature chunks for H
TT = GT // 128              # 128-token tiles per group
NTILES = G * TT             # 128-token tiles per core


def build_kernel(use_b3: bool, reps: int = 1):
    import concourse.bass as bass
    import concourse.mybir as mybir
    import concourse.tile as tile
    from concourse import bacc
    from concourse.masks import make_identity

    dt = mybir.dt
    AF = mybir.ActivationFunctionType
    ALU = mybir.AluOpType

    nc = bacc.Bacc("TRN2", target_bir_lowering=False, debug=False)

    xh_d = nc.dram_tensor("xh", [BL, DIN], dt.float16, kind="ExternalInput").ap()
    xl_d = nc.dram_tensor("xl", [BL, DIN], dt.float16, kind="ExternalInput").ap()
    w1_d = nc.dram_tensor("w1", [E, DIN, H], dt.float16, kind="ExternalInput").ap()
    w2_d = nc.dram_tensor("w2", [E, H, H], dt.float16, kind="ExternalInput").ap()
    w3_d = nc.dram_tensor("w3", [E, H, O], dt.float16, kind="ExternalInput").ap()
    wgh_d = nc.dram_tensor("wgh", [DIN, E], dt.float16, kind="ExternalInput").ap()
    wgl_d = nc.dram_tensor("wgl", [DIN, E], dt.float16, kind="ExternalInput").ap()
    b1_d = nc.dram_tensor("b1r", [128, E, HC], dt.float32, kind="ExternalInput").ap()
    b2_d = nc.dram_tensor("b2r", [128, E, HC], dt.float32, kind="ExternalInput").ap()
    b3_d = nc.dram_tensor("b3", [E, O], dt.float16, kind="ExternalInput").ap()
    out_d = nc.dram_tensor("out", [BL, O], dt.float32, kind="ExternalOutput").ap()
    sc_d = nc.dram_tensor("scores", [BL, E], dt.float32, kind="ExternalOutput").ap()
    st_d = nc.dram_tensor("stats", [16, 1], dt.float32, kind="ExternalOutput").ap()

    with tile.TileContext(nc) as tc:
        with (
            tc.tile_pool(name="const", bufs=1) as cpool,
            tc.tile_pool(name="xt2", bufs=2) as xtpool,
            tc.tile_pool(name="xt1", bufs=1) as xtpool1,
            tc.tile_pool(name="act", bufs=2) as apool,
            tc.tile_pool(name="sm", bufs=4) as smpool,
            tc.tile_pool(name="wvp", bufs=2 * TT) as wvpool,
            tc.tile_pool(name="pmm", bufs=2, space="PSUM") as pmm,
            tc.tile_pool(name="pmisc", bufs=1, space="PSUM") as pmisc,
        ):
            # ---- constants / weights (resident) ----
            ident = cpool.tile([128, 128], dt.float32)
            make_identity(nc, ident[:])

            w1s, w2s, w3s = [], [], []
            for e in range(E):
                t1 = cpool.tile([128, KO, H], dt.float16, tag=f"w1_{e}")
                nc.sync.dma_start(t1[:], w1_d[e].rearrange("(ko p) h -> p ko h", p=128))
                w1s.append(t1)
                t2 = cpool.tile([128, HC, H], dt.float16, tag=f"w2_{e}")
                nc.sync.dma_start(t2[:], w2_d[e].rearrange("(k p) h -> p k h", p=128))
                w2s.append(t2)
                t3 = cpool.tile([128, HC, O], dt.float16, tag=f"w3_{e}")
                nc.sync.dma_start(t3[:], w3_d[e].rearrange("(k p) o -> p k o", p=128))
                w3s.append(t3)
            # stacked gating weights: Wg_hi in psum partitions 0:E, Wg_lo at
            # 32:32+E (PSUM reads must be 32-aligned; the zero columns in
            # between cost nothing — matmul time is N cycles regardless of M)
            GB = 32
            wghl = cpool.tile([128, KO, GB + E], dt.float16)
            nc.vector.memset(wghl[:], 0.0)
            nc.sync.dma_start(
                wghl[:, :, 0:E], wgh_d.rearrange("(ko p) e -> p ko e", p=128))
            nc.sync.dma_start(
                wghl[:, :, GB:GB + E], wgl_d.rearrange("(ko p) e -> p ko e", p=128))
            b1s = cpool.tile([128, E, HC], dt.float32)
            nc.sync.dma_start(b1s[:], b1_d)
            b2s = cpool.tile([128, E, HC], dt.float32)
            nc.sync.dma_start(b2s[:], b2_d)
            if use_b3:
                b3s = cpool.tile([E, O], dt.float16)
                nc.sync.dma_start(b3s[:], b3_d)
                onesrow = cpool.tile([1, 128], dt.float16)
                nc.vector.memset(onesrow[:], 1.0)

            scstage = cpool.tile([128, NTILES, E], dt.float32)
            sacc = cpool.tile([128, 2 * E], dt.float32)

            def emit_rep():
                nc.vector.memset(sacc[:], 0.0)
                for g in range(G):
                    rows = slice(g * GT, (g + 1) * GT)
                    xthi = xtpool.tile([128, KO, GT], dt.float16, tag="xthi")
                    nc.sync.dma_start_transpose(xthi[:], xh_d[rows, :])
                    xtlo = xtpool1.tile([128, KO, GT], dt.float16, tag="xtlo")
                    nc.sync.dma_start_transpose(xtlo[:], xl_d[rows, :])

                    # ---- gating, full-precision fp16 split:
                    # psum rows 0:E   += Wg_hi.T @ (x_hi + x_lo)
                    # psum rows E:2E  += Wg_lo.T @ (x_hi + x_lo)
                    # logits = rows[0:E] + rows[E:2E], summed later by two
                    # accumulating PE-transposes. 24 matmuls instead of 36.
                    pl = pmisc.tile([GB + E, GT], dt.float32, tag="pl")
                    i = 0
                    for xt in (xthi, xtlo):
                        for ko in range(KO):
                            nc.tensor.matmul(
                                pl[:], wghl[:, ko, :], xt[:, ko, :],
                                start=(i == 0), stop=(i == 2 * KO - 1))
                            i += 1
                    lsbA = smpool.tile([E, GT], dt.float32, tag="lsbA")
                    nc.scalar.mul(lsbA[:], pl[0:E, :], 1.0 / TEMPERATURE)
                    lsbB = smpool.tile([E, GT], dt.float32, tag="lsbB")
                    nc.scalar.mul(lsbB[:], pl[GB:GB + E, :], 1.0 / TEMPERATURE)

                    wvs = []
                    for t in range(TT):
                        tile_idx = g * TT + t
                        tsl = slice(t * 128, (t + 1) * 128)
                        ptt = pmisc.tile([128, E], dt.float32, tag="ptt")
                        nc.tensor.matmul(
                            ptt[:], lsbA[:, tsl], ident[:E, :E],
                            is_transpose=True, start=True, stop=False)
                        nc.tensor.matmul(
                            ptt[:], lsbB[:, tsl], ident[:E, :E],
                            is_transpose=True, start=False, stop=True)
                        nmx = smpool.tile([128, 1], dt.float32, tag="nmx")
                        nc.vector.tensor_reduce(
                            nmx[:], ptt[:], axis=mybir.AxisListType.X,
                            op=ALU.max, negate=True,
                        )
                        es = smpool.tile([128, E], dt.float32, tag="es")
                        se = smpool.tile([128, 1], dt.float32, tag="se")
                        nc.scalar.activation(
                            es[:], ptt[:], AF.Exp, bias=nmx[:], scale=1.0,
                            accum_out=se[:],
                        )
                        rse = smpool.tile([128, 1], dt.float32, tag="rse")
                        nc.vector.reciprocal(rse[:], se[:])
                        sc = scstage[:, tile_idx, :]
                        nc.vector.tensor_scalar_mul(sc, es[:], rse[:])
                        # top-2 by masked max
                        s1 = smpool.tile([128, 1], dt.float32, tag="s1")
                        nc.vector.tensor_reduce(
                            s1[:], sc, axis=mybir.AxisListType.X, op=ALU.max)
                        m1 = smpool.tile([128, E], dt.float32, tag="m1")
                        nc.vector.tensor_scalar(m1[:], sc, s1[:], None, ALU.is_equal)
                        s2m = smpool.tile([128, E], dt.float32, tag="s2m")
                        nc.vector.scalar_tensor_tensor(
                            s2m[:], m1[:], -1e30, sc, op0=ALU.mult, op1=ALU.add)
                        s2 = smpool.tile([128, 1], dt.float32, tag="s2")
                        nc.vector.tensor_reduce(
                            s2[:], s2m[:], axis=mybir.AxisListType.X, op=ALU.max)
                        m2 = smpool.tile([128, E], dt.float32, tag="m2")
                        nc.vector.tensor_scalar(m2[:], s2m[:], s2[:], None, ALU.is_equal)
                        den = smpool.tile([128, 1], dt.float32, tag="den")
                        nc.vector.tensor_add(den[:], s1[:], s2[:])
                        rden = smpool.tile([128, 1], dt.float32, tag="rden")
                        nc.vector.reciprocal(rden[:], den[:])
                        g1 = smpool.tile([128, 1], dt.float32, tag="g1")
                        nc.vector.tensor_mul(g1[:], s1[:], rden[:])
                        g2 = smpool.tile([128, 1], dt.float32, tag="g2")
                        nc.vector.tensor_mul(g2[:], s2[:], rden[:])
                        wv = wvpool.tile([128, E], dt.float32, tag="wv")
                        nc.vector.tensor_scalar_mul(wv[:], m2[:], g2[:])
                        nc.vector.scalar_tensor_tensor(
                            wv[:], m1[:], g1[:], wv[:], op0=ALU.mult, op1=ALU.add)
                        wvs.append(wv)
                        m12 = smpool.tile([128, E], dt.float32, tag="m12")
                        nc.vector.tensor_add(m12[:], m1[:], m2[:])
                        # per-partition stats accumulation (RAW-chained on DVE;
                        # cross-tile PSUM-matmul accumulation is unsafe under
                        # scheduler reordering)
                        nc.vector.tensor_add(sacc[:, 0:E], sacc[:, 0:E], sc)
                        nc.vector.tensor_add(sacc[:, E:2 * E], sacc[:, E:2 * E], m12[:])

                    # ---- experts ----
                    acc = apool.tile([128, TT, O], dt.float32, tag="acc")
                    for e in range(E):
                        h1 = apool.tile([128, HC, GT], dt.float16, tag="h1")
                        for hc in range(HC):
                            ph = pmm.tile([128, GT], dt.float32, tag="ph")
                            for ko in range(KO):
                                nc.tensor.matmul(
                                    ph[:], w1s[e][:, ko, hc * 128:(hc + 1) * 128],
                                    xthi[:, ko, :],
                                    start=(ko == 0), stop=(ko == KO - 1))
                            nc.scalar.activation(
                                h1[:, hc, :], ph[:], AF.Relu,
                                bias=b1s[:, e, hc:hc + 1], scale=1.0)
                        h2 = apool.tile([128, HC, GT], dt.float16, tag="h2")
                        for oc in range(HC):
                            ph = pmm.tile([128, GT], dt.float32, tag="ph")
                            for k in range(HC):
                                nc.tensor.matmul(
                                    ph[:], w2s[e][:, k, oc * 128:(oc + 1) * 128],
                                    h1[:, k, :],
                                    start=(k == 0), stop=(k == HC - 1))
                            nc.scalar.activation(
                                h2[:, oc, :], ph[:], AF.Relu,
                                bias=b2s[:, e, oc:oc + 1], scale=1.0)
                        for t in range(TT):
                            tsl = slice(t * 128, (t + 1) * 128)
                            po = pmm.tile([128, O], dt.float32, tag="po")
                            for k in range(HC):
                                nc.tensor.matmul(
                                    po[:], h2[:, k, tsl], w3s[e][:, k, :],
                                    start=(k == 0),
                                    stop=(k == HC - 1 and not use_b3))
                            if use_b3:
                                nc.tensor.matmul(
                                    po[:], onesrow[:], b3s[e:e + 1, :],
                                    start=False, stop=True)
                            wsl = wvs[t][:, e:e + 1]
                            if e == 0:
                                nc.scalar.activation(
                                    acc[:, t, :], po[:], AF.Copy,
                                    bias=0.0, scale=wsl)
                            else:
                                nc.vector.scalar_tensor_tensor(
                                    acc[:, t, :], po[:], wsl, acc[:, t, :],
                                    op0=ALU.mult, op1=ALU.add)

                    nc.sync.dma_start(
                        out_d.rearrange("(g t p) o -> g p t o", t=TT, p=128)[g],
                        acc[:])

                # ---- epilogue (per rep; cheap): partition-reduce the stats
                pst = pmisc.tile([2 * E, 128], dt.float32, tag="pst")
                nc.tensor.transpose(pst[:], sacc[:], ident[:])
                stsb = smpool.tile([2 * E, 128], dt.float32, tag="stsb")
                nc.scalar.copy(stsb[:], pst[:])
                stq = smpool.tile([2 * E, 1], dt.float32, tag="stq")
                nc.vector.tensor_reduce(
                    stq[:], stsb[:], axis=mybir.AxisListType.X, op=ALU.add)
                nc.sync.dma_start(st_d[:2 * E], stq[:])
                nc.sync.dma_start(
                    sc_d.rearrange("(n p) e -> p n e", p=128), scstage[:])

            if reps == 1:
                emit_rep()
            else:
                with tc.For_i(0, reps, 1):
                    emit_rep()

    nc.compile()
    return nc


class _SpmdRunner:
    """Compile once; run the SPMD kernel on 8 cores with sharded inputs."""

    def __init__(self, nc, n_cores=NCORES):
        import jax
        import concourse.mybir as mybir
        from jax.sharding import Mesh, PartitionSpec, NamedSharding
        from jax.experimental.shard_map import shard_map
        from concourse import bass2jax
        from concourse.bass2jax import _bass_exec_p, install_neuronx_cc_hook

        install_neuronx_cc_hook()
        self.jax = jax
        self.nc = nc
        self.n_cores = n_cores
        in_names, out_names, out_avals = [], [], []
        partition_name = (
            nc.partition_id_tensor.name if nc.partition_id_tensor else None)
        for alloc in nc.m.functions[0].allocations:
            if not isinstance(alloc, mybir.MemoryLocationSet):
                continue
            name = alloc.memorylocations[0].name
            if alloc.kind == "ExternalInput":
                if name != partition_name:
                    in_names.append(name)
            elif alloc.kind == "ExternalOutput":
                out_names.append(name)
                out_avals.append(jax.core.ShapedArray(
                    tuple(alloc.tensor_shape), mybir.dt.np(alloc.dtype)))
        self.in_names, self.out_names, self.out_avals = (
            in_names, out_names, out_avals)

        n_params = len(in_names)
        all_in_names = list(in_names) + list(out_names)
        if partition_name is not None:
            all_in_names.append(partition_name)

        def _body(*args):
            operands = list(args)
            if partition_name is not None:
                operands.append(bass2jax.partition_id_tensor())
            outs = _bass_exec_p.bind(
                *operands,
                out_avals=tuple(out_avals),
                in_names=tuple(all_in_names),
                out_names=tuple(out_names),
                lowering_input_output_aliases=(),
                sim_require_finite=True,
                sim_require_nnan=True,
                nc=nc,
            )
            return tuple(outs)

        devices = jax.devices()[:n_cores]
        mesh = Mesh(np.asarray(devices), ("core",))
        spec = PartitionSpec("core")
        self.sharding = NamedSharding(mesh, spec)
        self.fn = jax.jit(
            shard_map(_body, mesh=mesh, in_specs=(spec,) * (n_params + len(out_names)),
                      out_specs=(spec,) * len(out_names), check_rep=False),
            keep_unused=True,
        )
        import jax.numpy as jnp
        zero_shapes = [
            ((n_cores * av.shape[0],) + tuple(av.shape[1:]), av.dtype)
            for av in out_avals]

        def _zeros():
            return tuple(jnp.zeros(s, d) for s, d in zero_shapes)

        self.zfn = jax.jit(_zeros, out_shardings=(spec and self.sharding,) * len(zero_shapes))

    def put_inputs(self, in_maps):
        arrs = []
        for name in self.in_names:
            cat = np.concatenate([m[name] for m in in_maps], axis=0)
            arrs.append(self.jax.device_put(cat, self.sharding))
        return arrs

    def run(self, dev_inputs):
        return self.fn(*dev_inputs, *self.zfn())

    def results(self, outs):
        res = [dict() for _ in range(self.n_cores)]
        for i, name in enumerate(self.out_names):
            g = np.asarray(outs[i]).reshape(
                self.n_cores, *self.out_avals[i].shape)
            for c in range(self.n_cores):
                res[c][name] = g[c]
        return res


_RUNNERS = {}


def _get_runner(use_b3: bool, reps: int = 1):
    key = (use_b3, reps)
    if key not in _RUNNERS:
        nc = build_kernel(use_b3, reps)
        _RUNNERS[key] = _SpmdRunner(nc)
    return _RUNNERS[key]


def _prep_inputs(x, Wg, W1, b1, W2, b2, W3, b3):
    f16, f32 = np.float16, np.float32
    xh = x.astype(f16)
    xl = (x - xh.astype(f32)).astype(f16)
    wgh = Wg.astype(f16)
    wgl = (Wg - wgh.astype(f32)).astype(f16)
    w1h, w2h, w3h = W1.astype(f16), W2.astype(f16), W3.astype(f16)
    b1r = np.ascontiguousarray(
        b1.reshape(E, HC, 128).transpose(2, 0, 1).astype(f32))
    b2r = np.ascontiguousarray(
        b2.reshape(E, HC, 128).transpose(2, 0, 1).astype(f32))
    b3h = b3.astype(f16)
    in_maps = []
    for c in range(NCORES):
        rows = slice(c * BL, (c + 1) * BL)
        in_maps.append({
            "xh": np.ascontiguousarray(xh[rows]),
            "xl": np.ascontiguousarray(xl[rows]),
            "w1": w1h, "w2": w2h, "w3": w3h,
            "wgh": wgh, "wgl": wgl,
            "b1r": b1r, "b2r": b2r, "b3": b3h,
        })
    return in_maps


def kernel(x, Wg, W1, b1, W2, b2, W3, b3, top_k):
    x = np.asarray(x, np.float32)
    Wg = np.asarray(Wg, np.float32)
    W1 = np.asarray(W1, np.float32)
    b1 = np.asarray(b1, np.float32)
    W2 = np.asarray(W2, np.float32)
    b2 = np.asarray(b2, np.float32)
    W3 = np.asarray(W3, np.float32)
    b3 = np.asarray(b3, np.float32)
    assert int(top_k) == 2, "kernel hardcodes top_k=2"

    use_b3 = bool(np.any(b3))
    runner = _get_runner(use_b3)
    in_maps = _prep_inputs(x, Wg, W1, b1, W2, b2, W3, b3)
    dev_in = runner.put_inputs(in_maps)
    outs = runner.run(dev_in)
    res = runner.results(outs)

    out = np.concatenate([res[c]["out"] for c in range(NCORES)], axis=0)
    scores = np.concatenate([res[c]["scores"] for c in range(NCORES)], axis=0)
    stats = np.sum([res[c]["stats"][:, 0] for c in range(NCORES)], axis=0)
    mean_prob = stats[0:E] / B
    frac = stats[E:2 * E] / B
    lb_loss = np.float32(LB_WEIGHT * E * np.dot(mean_prob, frac))
    return out, lb_loss, scores
